# revision 33
# baseline (speedup 1.0000x reference)
"""Trainium2 Bass kernel for nn_EnokeeEncoder (ragged mention pooling +
4-layer transformer + 50k-entity classifier), data-parallel over batch
across 8 NeuronCores.

Layout strategy per core (2 batches, 256 mention-tokens):
  - residual stream x: token-major [128 tokens/p, 768] bf16 (LN/softmax easy)
  - matmul chains run feature-major via PE transposes of x
  - all big matmuls in bf16 (weights pre-cast+pre-transposed on host),
    f32 accumulation in PSUM; LN / softmax / residual in f32.
  - residual adds on DVE straight off PSUM (no PE identity matmuls)
  - attention: scores^T per head for both batches in one psum tile, one
    exp; softmax denominator via a fused ones-column in an augmented V
  - ff1 -> relu -> ff2 merged into one per-k-tile stream (small hT pool)
  - classifier bias folded into an augmented K=101 contraction row;
    w2 streams on the gpsimd DMA queue, paced by per-layer markers so
    it fills HBM bandwidth the layer weight stream leaves idle.
"""

import sys

for _p in ("/opt/trn_rl_repo",):
    if _p not in sys.path:
        sys.path.insert(0, _p)

import numpy as np
import ml_dtypes

BF16 = ml_dtypes.bfloat16

B, M, L, S = 16, 128, 32, 512
D, H, DFF, NL = 768, 12, 3072, 4
NE = 50000
HD = D // H
EPS = 1e-5
N_CORES = 8
BL = B // N_CORES          # batches per core
P = 128
KD = D // P                # 6 k-tiles over D
KF = DFF // P              # 24 k-tiles over DFF
MT = BL                    # token m-tiles per core (M == P)
NQK = 2 * D // P           # 12 m-tiles over q,k features
VW = 68                    # per-head stride in augmented V (64 v + 1 one + pad)

KERNEL_DEBUG = False
_CACHE = {}


def _build(cfg):
    (attn_b_val, vb_nz, outb_nz, ff2b_nz, ln1_nt, ln2_nt, debug) = cfg
    from contextlib import ExitStack

    import concourse.bass as bass
    import concourse.bacc as bacc
    import concourse.mybir as mybir
    import concourse.tile as tile
    from concourse.masks import make_identity

    dt = mybir.dt
    AF = mybir.ActivationFunctionType
    OP = mybir.AluOpType
    AX = mybir.AxisListType
    f32 = dt.float32
    bf16 = dt.bfloat16

    nc = bacc.Bacc("TRN2", target_bir_lowering=False, debug=False,
                   enable_asserts=False, num_devices=N_CORES)

    # ---- DRAM I/O ----
    lhs32_d = nc.dram_tensor("lhs32", [BL, L, D], f32, kind="ExternalInput").ap()
    vmT_d = nc.dram_tensor("vmT", [BL, L, M], f32, kind="ExternalInput").ap()
    attnw_d = nc.dram_tensor("attnw", [L, D], f32, kind="ExternalInput").ap()
    qkvw_d = nc.dram_tensor("qkvw", [NL, KD, P, 3 * D], bf16, kind="ExternalInput").ap()
    qkvb_d = nc.dram_tensor("qkvb", [P, NL, 2 * KD], f32, kind="ExternalInput").ap()
    outw_d = nc.dram_tensor("outw", [NL, KD, P, D], bf16, kind="ExternalInput").ap()
    ff1w_d = nc.dram_tensor("ff1w", [NL, KD, P, DFF], bf16, kind="ExternalInput").ap()
    ff1b_d = nc.dram_tensor("ff1b", [P, NL, KF], f32, kind="ExternalInput").ap()
    ff2w_d = nc.dram_tensor("ff2w", [NL, KF, P, D], bf16, kind="ExternalInput").ap()
    w1T_d = nc.dram_tensor("w1T", [KD, P, 100], bf16, kind="ExternalInput").ap()
    CH = 2048
    NCH = (NE + CH - 1) // CH
    w2a_d = nc.dram_tensor("w2a", [NCH, P, CH], bf16, kind="ExternalInput").ap()
    outb_d = ff2b_d = None
    ln1w_d = ln1b_d = ln2w_d = ln2b_d = None
    if outb_nz:
        outb_d = nc.dram_tensor("outb", [NL, D], f32, kind="ExternalInput").ap()
    if ff2b_nz:
        ff2b_d = nc.dram_tensor("ff2b", [NL, D], f32, kind="ExternalInput").ap()
    if ln1_nt:
        ln1w_d = nc.dram_tensor("ln1w", [NL, D], f32, kind="ExternalInput").ap()
        ln1b_d = nc.dram_tensor("ln1b", [NL, D], f32, kind="ExternalInput").ap()
    if ln2_nt:
        ln2w_d = nc.dram_tensor("ln2w", [NL, D], f32, kind="ExternalInput").ap()
        ln2b_d = nc.dram_tensor("ln2b", [NL, D], f32, kind="ExternalInput").ap()
    vbias_d = None
    if vb_nz:
        vbias_d = nc.dram_tensor("vbias", [NL, D], f32, kind="ExternalInput").ap()
    out_d = nc.dram_tensor("out", [NCH, BL, P, CH], bf16, kind="ExternalOutput").ap()
    xdbg_d = None
    if debug:
        xdbg_d = nc.dram_tensor("xdbg", [NL + 1, BL, M, D], f32,
                                kind="ExternalOutput").ap()

    def bcast_ap(ap, parts):
        return bass.AP(tensor=ap.tensor, offset=ap.offset,
                       ap=[[0, parts]] + [list(x) for x in ap.ap])

    def strided_view(ap, blocks, blk_stride, width):
        """[P, blocks*width] view with per-block stride over a 2D tile AP."""
        return bass.AP(tensor=ap.tensor, offset=ap.offset,
                       ap=[list(ap.ap[0]), [blk_stride, blocks], [1, width]])

    with tile.TileContext(nc) as tc, ExitStack() as ctx:
        const = ctx.enter_context(tc.tile_pool(name="const", bufs=1))
        pools = ctx.enter_context(tc.tile_pool(name="pools", bufs=2))
        xpool = ctx.enter_context(tc.tile_pool(name="xpool", bufs=2))
        xTp = ctx.enter_context(tc.tile_pool(name="xTp", bufs=14))
        qkTp = ctx.enter_context(tc.tile_pool(name="qkTp", bufs=13))
        aTp = ctx.enter_context(tc.tile_pool(name="aTp", bufs=4))
        aop = ctx.enter_context(tc.tile_pool(name="aop", bufs=13))
        aoTp = ctx.enter_context(tc.tile_pool(name="aoTp", bufs=7))
        hTp = ctx.enter_context(tc.tile_pool(name="hTp", bufs=12))
        vp = ctx.enter_context(tc.tile_pool(name="vp", bufs=2))
        xbp = ctx.enter_context(tc.tile_pool(name="xbp", bufs=3))
        stat = ctx.enter_context(tc.tile_pool(name="stat", bufs=12))
        wq = ctx.enter_context(tc.tile_pool(name="wq", bufs=6))
        wo = ctx.enter_context(tc.tile_pool(name="wo", bufs=6))
        wf1 = ctx.enter_context(tc.tile_pool(name="wf1", bufs=6))
        wf2 = ctx.enter_context(tc.tile_pool(name="wf2", bufs=12))
        w2p = ctx.enter_context(tc.tile_pool(name="w2p", bufs=10))
        ostp = ctx.enter_context(tc.tile_pool(name="ostp", bufs=3))
        mkp = ctx.enter_context(tc.tile_pool(name="mkp", bufs=2))
        psA = ctx.enter_context(tc.tile_pool(name="psA", bufs=4, space="PSUM"))
        psB = ctx.enter_context(tc.tile_pool(name="psB", bufs=2, space="PSUM"))

        # ---- constants ----
        idb = const.tile([P, P], bf16, tag="idb", name="idb")
        make_identity(nc, idb[:])
        ones32 = const.tile([L, 1], f32, tag="ones32", name="ones32")
        nc.vector.memset(ones32[:], 1.0)
        epst = const.tile([P, 1], f32, tag="epst", name="epst")
        nc.vector.memset(epst[:], EPS)
        lhs32_sb = const.tile([L, BL, D], f32, tag="lhs32", name="lhs32")
        vmT_sb = const.tile([L, BL, M], f32, tag="vmT", name="vmT")
        for b in range(BL):
            nc.sync.dma_start(out=lhs32_sb[:, b, :], in_=lhs32_d[b])
            nc.sync.dma_start(out=vmT_sb[:, b, :], in_=vmT_d[b])
        attnw_sb = const.tile([L, D], f32, tag="attnw", name="attnw")
        nc.sync.dma_start(out=attnw_sb[:], in_=attnw_d)
        qkvb_sb = const.tile([P, NL, 2 * KD], f32, tag="qkvb", name="qkvb")
        nc.sync.dma_start(out=qkvb_sb[:], in_=qkvb_d)
        ff1b_sb = const.tile([P, NL, KF], f32, tag="ff1b", name="ff1b")
        nc.sync.dma_start(out=ff1b_sb[:], in_=ff1b_d)
        w1T_sb = const.tile([P, KD, 100], bf16, tag="w1T", name="w1T")
        for ko in range(KD):
            nc.sync.dma_start(out=w1T_sb[:, ko, :], in_=w1T_d[ko])

        # ---- mention pooling ----
        x_t = [xpool.tile([P, D], f32, tag="x", name="x") for _ in range(MT)]
        lhsb = const.tile([L, BL, D], bf16, tag="lhsb", name="lhsb")
        for b in range(BL):
            nc.scalar.copy(lhsb[:, b, :], lhs32_sb[:, b, :])
        for b in range(BL):
            tmp = pools.tile([L, D], f32, tag="ptmp", name="ptmp")
            nc.vector.tensor_mul(tmp[:], lhs32_sb[:, b, :], attnw_sb[:])
            u = stat.tile([L, 1], f32, tag="u", name="u")
            nc.vector.tensor_reduce(u[:], tmp[:], axis=AX.X, op=OP.add)
            expT = pools.tile([L, M], f32, tag="pexp", name="pexp")
            nc.scalar.activation(expT[:], vmT_sb[:, b, :], AF.Exp,
                                 bias=float(attn_b_val), scale=u[:])
            wun = pools.tile([L, M], bf16, tag="pwun", name="pwun")
            nc.vector.tensor_mul(wun[:], expT[:], vmT_sb[:, b, :])
            ps_d = psA.tile([P, 256], f32, tag="s", name="s")
            nc.tensor.matmul(ps_d[:, 0:1], expT[:], ones32[:], start=True, stop=True)
            r = stat.tile([P, 1], f32, tag="r", name="r")
            nc.vector.reciprocal(r[:], ps_d[:, 0:1])
            ps_x = psB.tile([P, D], f32, tag="w", name="w")
            for n0, n1 in ((0, 512), (512, D)):
                nc.tensor.matmul(ps_x[:, n0:n1], wun[:], lhsb[:, b, n0:n1],
                                 start=True, stop=True)
            nc.vector.tensor_scalar_mul(x_t[b][:], ps_x[:], r[:])

        if debug:
            for b in range(BL):
                nc.sync.dma_start(out=xdbg_d[0, b], in_=x_t[b][:])

        # ---- helpers ----
        _tct = [0]

        def ptranspose(dst_ap, src_ap):
            """bf16 [128,128] transpose via PE into psum, evicted to dst."""
            ps = psA.tile([P, P], bf16, tag="s", name="t")
            nc.tensor.transpose(ps[:], src_ap, idb[:])
            _tct[0] += 1
            if _tct[0] % 2 == 0:
                nc.scalar.copy(dst_ap, ps[:])
            else:
                nc.vector.tensor_copy(dst_ap, ps[:])
            return ps

        def layernorm_one(xin, w_bc, b_bc, want_f32):
            """LN over a [P, D] SBUF f32 view. Returns (xo f32 or None,
            xb bf16)."""
            st = stat.tile([P, 2, 6], f32, tag="bns", name="bns")
            for s in range(2):
                nc.vector.bn_stats(st[:, s, :], xin[:, s * 384:(s + 1) * 384])
            mv = stat.tile([P, 2], f32, tag="mv", name="mv")
            nc.vector.bn_aggr(mv[:], st[:])
            std = stat.tile([P, 1], f32, tag="sd", name="sd")
            nc.scalar.activation(std[:], mv[:, 1:2], AF.Sqrt,
                                 bias=epst[:], scale=1.0)
            rstd = stat.tile([P, 1], f32, tag="rs", name="rs")
            nc.vector.reciprocal(rstd[:], std[:])
            nms = stat.tile([P, 1], f32, tag="ns", name="ns")
            nc.vector.tensor_scalar(nms[:], mv[:, 0:1], rstd[:], -1.0,
                                    op0=OP.mult, op1=OP.mult)
            xo = None
            if want_f32 or w_bc is not None or b_bc is not None:
                xo = xpool.tile([P, D], f32, tag="x", name="x")
                for s in range(2):
                    nc.vector.tensor_scalar(xo[:, s * 384:(s + 1) * 384],
                                            xin[:, s * 384:(s + 1) * 384],
                                            rstd[:], nms[:],
                                            op0=OP.mult, op1=OP.add)
                if w_bc is not None:
                    nc.vector.tensor_mul(xo[:], xo[:], w_bc[:])
                if b_bc is not None:
                    nc.vector.tensor_add(xo[:], xo[:], b_bc[:])
            xb = xbp.tile([P, D], bf16, tag="xb", name="xb")
            if w_bc is None and b_bc is None:
                # split the normalize-eviction across ACT and DVE so the
                # serial LN chain is ~500ns shorter and the first half is
                # ready for its transposes earlier
                nc.scalar.activation(xb[:, 0:384], xin[:, 0:384], AF.Identity,
                                     bias=nms[:], scale=rstd[:])
                nc.vector.tensor_scalar(xb[:, 384:D], xin[:, 384:D],
                                        rstd[:], nms[:],
                                        op0=OP.mult, op1=OP.add)
            else:
                nc.scalar.copy(xb[:], xo[:])
            return xo, xb

        # classifier w2 prefetch: [P, CH] bf16 chunks on the gpsimd DMA
        # queue, released in per-layer tranches behind marker reads of
        # that layer's last ff2w tile (so w2 streams in the bandwidth the
        # weight stream leaves idle, never ahead of it).
        w2_tiles = {}

        def load_w2(ci, eng=None):
            if ci < NCH and ci not in w2_tiles:
                w2t = w2p.tile([P, CH], bf16, tag="w2", name="w2")
                (eng or nc.gpsimd).dma_start(out=w2t[:], in_=w2a_d[ci])
                w2_tiles[ci] = w2t

        def w2_marker(gate_ap):
            mk = mkp.tile([1, 8], bf16, tag="mk", name="mk")
            nc.gpsimd.dma_start(out=mk[:], in_=gate_ap)

        xb_t = []
        for mo in range(MT):
            xb = xbp.tile([P, D], bf16, tag="xb", name="xb")
            if mo % 2 == 0:
                nc.scalar.copy(xb[:], x_t[mo][:])
            else:
                nc.vector.tensor_copy(xb[:], x_t[mo][:])
            xb_t.append(xb)

        W2_TRANCHE = 3
        w2_next = [0]

        def release_w2(n):
            for _ in range(n):
                if w2_next[0] < NCH:
                    load_w2(w2_next[0])
                    w2_next[0] += 1

        # ---- transformer layers ----
        for i in range(NL):
            qkvw_t = [wq.tile([P, 3 * D], bf16, tag="qkvw", name="qkvw") for _ in range(KD)]
            wde = nc.sync
            for ko in range(KD):
                wde.dma_start(out=qkvw_t[ko][:], in_=qkvw_d[i, ko])
            outw_t = [wo.tile([P, D], bf16, tag="outw", name="outw") for _ in range(KD)]
            for ko in range(KD):
                wde.dma_start(out=outw_t[ko][:], in_=outw_d[i, ko])
            ff1w_t = [wf1.tile([P, DFF], bf16, tag="ff1w", name="ff1w") for _ in range(KD)]
            for ko in range(KD):
                nc.sync.dma_start(out=ff1w_t[ko][:], in_=ff1w_d[i, ko])
            ff2w_t = [wf2.tile([P, D], bf16, tag="ff2w", name="ff2w") for _ in range(KF)]
            for ko in range(KF):
                nc.sync.dma_start(out=ff2w_t[ko][:], in_=ff2w_d[i, ko])
            # release a w2 tranche once this layer's ff2w stream is near done
            w2_marker(ff2w_t[KF - 1][0:1, 0:8])
            release_w2(W2_TRANCHE if i < NL - 1 else w2p.bufs)
            vb_bc = None
            if vb_nz:
                vb_bc = pools.tile([P, D], f32, tag="vbb", name="vbb")
                nc.gpsimd.dma_start(out=vb_bc[:],
                                    in_=bcast_ap(vbias_d[i], P))
            outb_bc = None
            if outb_nz:
                outb_bc = pools.tile([P, D], f32, tag="obb", name="obb")
                nc.gpsimd.dma_start(out=outb_bc[:], in_=bcast_ap(outb_d[i], P))
            ff2b_bc = None
            if ff2b_nz:
                ff2b_bc = pools.tile([P, D], f32, tag="fbb", name="fbb")
                nc.gpsimd.dma_start(out=ff2b_bc[:], in_=bcast_ap(ff2b_d[i], P))
            ln1w_bc = ln1b_bc = ln2w_bc = ln2b_bc = None
            if ln1_nt:
                ln1w_bc = pools.tile([P, D], f32, tag="l1w", name="l1w")
                nc.gpsimd.dma_start(out=ln1w_bc[:], in_=bcast_ap(ln1w_d[i], P))
                ln1b_bc = pools.tile([P, D], f32, tag="l1b", name="l1b")
                nc.gpsimd.dma_start(out=ln1b_bc[:], in_=bcast_ap(ln1b_d[i], P))
            if ln2_nt:
                ln2w_bc = pools.tile([P, D], f32, tag="l2w", name="l2w")
                nc.gpsimd.dma_start(out=ln2w_bc[:], in_=bcast_ap(ln2w_d[i], P))
                ln2b_bc = pools.tile([P, D], f32, tag="l2b", name="l2b")
                nc.gpsimd.dma_start(out=ln2b_bc[:], in_=bcast_ap(ln2b_d[i], P))

            # xT transposes + v (per m-tile tiles so deps are exact: m0's
            # transposes/v start as soon as LN2(m0) is done, covering the
            # m1 LN chain with PE work)
            xT = [[xTp.tile([P, P], bf16, tag="xT", name="xT")
                   for _ in range(KD)] for _ in range(MT)]
            v_aug = []
            for mo in range(MT):
                for ko in range(KD):
                    ptranspose(xT[mo][ko][:],
                               xb_t[mo][:, ko * P:(ko + 1) * P])
                ps_v = psB.tile([P, D], f32, tag="w", name="w")
                for ko in range(KD):
                    for n0, n1 in ((0, 512), (512, D)):
                        nc.tensor.matmul(
                            ps_v[:, n0:n1], xT[mo][ko][:],
                            qkvw_t[ko][:, 2 * D + n0:2 * D + n1],
                            start=(ko == 0), stop=(ko == KD - 1))
                va = vp.tile([P, H * VW], bf16, tag="v", name="v")
                nc.vector.memset(va[:], 1.0)
                va_view = strided_view(va[:], H, VW, HD)
                ps_view = strided_view(ps_v[:], H, HD, HD)
                if vb_nz:
                    nc.vector.tensor_add(ps_v[:], ps_v[:], vb_bc[:])
                nc.scalar.copy(va_view, ps_view)
                v_aug.append(va)

            # q,k feature-major [1536, 256]
            qkT = []
            for mo12 in range(NQK):
                ps = psA.tile([P, 256], f32, tag="s", name="s")
                # m-tiles accumulate sequentially: a start=True clears the
                # whole bank's has_written bits, so sub-groups must not
                # interleave
                for mo in range(MT):
                    for ko in range(KD):
                        nc.tensor.matmul(ps[:, mo * P:(mo + 1) * P],
                                         qkvw_t[ko][:, mo12 * P:(mo12 + 1) * P],
                                         xT[mo][ko][:],
                                         start=(ko == 0), stop=(ko == KD - 1))
                t = qkTp.tile([P, 256], bf16, tag="qkT", name="qkT")
                if mo12 % 2 == 0:
                    nc.scalar.activation(t[:], ps[:], AF.Identity,
                                         bias=qkvb_sb[:, i, mo12:mo12 + 1], scale=1.0)
                else:
                    nc.vector.tensor_scalar_add(t[:], ps[:],
                                                qkvb_sb[:, i, mo12:mo12 + 1])
                qkT.append(t)

            # attention: per head, S^T for both batches into one psum tile,
            # one exp -> aT; AV against augmented V gives ao^T-input plus the
            # softmax denominator in one matmul; out-proj accumulates per
            # head-pair as soon as its transposes are done.
            ps_o = [psB.tile([P, D], f32, tag="w", name="w") for _ in range(MT)]
            aT = {}
            ao_pair = [[None] * (H // 2) for _ in range(MT)]
            scale = 1.0 / np.sqrt(HD)

            def attn_scores(h):
                t_idx, row0 = h // 2, (h % 2) * HD
                ps_s = psA.tile([P, 256], f32, tag="s", name="s")
                for b in range(MT):
                    q_ap = qkT[t_idx][row0:row0 + HD, b * P:(b + 1) * P]
                    k_ap = qkT[KD + t_idx][row0:row0 + HD, b * P:(b + 1) * P]
                    nc.tensor.matmul(ps_s[:, b * P:(b + 1) * P], k_ap, q_ap,
                                     start=True, stop=True)
                a = aTp.tile([P, 256], bf16, tag="aT", name="aT")
                nc.scalar.activation(a[:], ps_s[:], AF.Exp, scale=scale)
                aT[h] = a

            def attn_av(h):
                t_idx = h // 2
                ps_av = psA.tile([P, 256], f32, tag="s", name="s")
                for b in range(MT):
                    nc.tensor.matmul(
                        ps_av[:, b * P:b * P + HD + 1], aT[h][:, b * P:(b + 1) * P],
                        v_aug[b][:, h * VW:h * VW + HD + 1],
                        start=True, stop=True)
                for b in range(MT):
                    if ao_pair[b][t_idx] is None:
                        ao_pair[b][t_idx] = aop.tile([P, P], bf16, tag="aop",
                                                     name="aop")
                    rec = stat.tile([P, 1], f32, tag="rc", name="rc")
                    dst = ao_pair[b][t_idx][:, (h % 2) * HD:(h % 2 + 1) * HD]
                    nc.vector.reciprocal(rec[:], ps_av[:, b * P + HD:b * P + HD + 1])
                    if (h + b) % 2 == 0:
                        nc.scalar.activation(dst, ps_av[:, b * P:b * P + HD],
                                             AF.Identity, scale=rec[:])
                    else:
                        nc.vector.tensor_scalar_mul(dst, ps_av[:, b * P:b * P + HD],
                                                    rec[:])
                if h % 2 == 1:
                    aoT = aoTp.tile([P, 256], bf16, tag="aoT", name="aoT")
                    for b in range(MT):
                        ptranspose(aoT[:, b * P:(b + 1) * P], ao_pair[b][t_idx][:])
                    aoT_t.append(aoT)
                    # out-proj for m0 accumulates inline; m1's is deferred
                    # so psO[0] completes ~2us early and its LN1 chain is
                    # covered by m1's out-proj matmuls
                    for n0, n1 in ((0, 512), (512, D)):
                        nc.tensor.matmul(
                            ps_o[0][:, n0:n1], aoT[:, 0:P],
                            outw_t[t_idx][:, n0:n1],
                            start=(t_idx == 0), stop=(t_idx == KD - 1))

            # software pipeline: scores/exp run two heads ahead of AV so
            # psum slot reuse follows consumption order (no cycles)
            aoT_t = []
            AHEAD = 2
            for h in range(H):
                attn_scores(h)
                if h >= AHEAD:
                    attn_av(h - AHEAD)
            for h in range(H - AHEAD, H):
                attn_av(h)
            for t_idx in range(KD):
                for n0, n1 in ((0, 512), (512, D)):
                    nc.tensor.matmul(
                        ps_o[1][:, n0:n1], aoT_t[t_idx][:, P:2 * P],
                        outw_t[t_idx][:, n0:n1],
                        start=(t_idx == 0), stop=(t_idx == KD - 1))

            # residual (DVE, off psum) + LN1, staggered per m-tile
            x1b_t, x1n_t = [], []
            for mo in range(MT):
                x1 = xpool.tile([P, D], f32, tag="x", name="x")
                if outb_nz:
                    nc.vector.scalar_tensor_tensor(
                        x1[:], ps_o[mo][:], 1.0, outb_bc[:],
                        op0=OP.mult, op1=OP.add)
                    nc.vector.tensor_add(x1[:], x1[:], xb_t[mo][:])
                else:
                    for s0, s1 in ((0, 384), (384, D)):
                        nc.vector.tensor_add(x1[:, s0:s1], ps_o[mo][:, s0:s1],
                                             xb_t[mo][:, s0:s1])
                xo, xb = layernorm_one(x1[:], ln1w_bc, ln1b_bc, want_f32=False)
                x1n_t.append(xo)
                x1b_t.append(xb)

            x1nT = [[xTp.tile([P, P], bf16, tag="xT", name="xT")
                     for _ in range(KD)] for _ in range(MT)]
            for mo in range(MT):
                for ko in range(KD):
                    ptranspose(x1nT[mo][ko][:],
                               x1b_t[mo][:, ko * P:(ko + 1) * P])

            # merged ff1(relu) -> ff2 stream, per DFF k-tile; the last TAIL
            # groups defer m1's ff2 so psY[0] completes early and LN2(m0)
            # is covered by m1's remaining matmuls
            ps_y = [psB.tile([P, D], f32, tag="w", name="w") for _ in range(MT)]
            TAIL = 9

            def ff1_group(ko24):
                ps = psA.tile([P, 256], f32, tag="s", name="s")
                for mo in range(MT):
                    for ko in range(KD):
                        nc.tensor.matmul(
                            ps[:, mo * P:(mo + 1) * P],
                            ff1w_t[ko][:, ko24 * P:(ko24 + 1) * P],
                            x1nT[mo][ko][:],
                            start=(ko == 0), stop=(ko == KD - 1))
                hT = hTp.tile([P, 256], bf16, tag="hT", name="hT")
                if ko24 % 2 == 0:
                    nc.scalar.activation(hT[:], ps[:], AF.Relu,
                                         bias=ff1b_sb[:, i, ko24:ko24 + 1], scale=1.0)
                else:
                    nc.vector.tensor_scalar(hT[:], ps[:],
                                            ff1b_sb[:, i, ko24:ko24 + 1], 0.0,
                                            op0=OP.add, op1=OP.max)
                return hT

            def ff2_group(ko24, hT, mo):
                for n0, n1 in ((0, 512), (512, D)):
                    nc.tensor.matmul(
                        ps_y[mo][:, n0:n1], hT[:, mo * P:(mo + 1) * P],
                        ff2w_t[ko24][:, n0:n1],
                        start=(ko24 == 0), stop=(ko24 == KF - 1))

            deferred = []
            for ko24 in range(KF):
                hT = ff1_group(ko24)
                ff2_group(ko24, hT, 0)
                if ko24 < KF - TAIL:
                    ff2_group(ko24, hT, 1)
                else:
                    deferred.append((ko24, hT))
            for ko24, hT in deferred:
                ff2_group(ko24, hT, 1)

            xb_t = []
            for mo in range(MT):
                x2 = xpool.tile([P, D], f32, tag="x", name="x")
                if ff2b_nz:
                    nc.vector.scalar_tensor_tensor(
                        x2[:], ps_y[mo][:], 1.0, ff2b_bc[:],
                        op0=OP.mult, op1=OP.add)
                    nc.vector.tensor_add(x2[:], x2[:], x1b_t[mo][:])
                else:
                    for s0, s1 in ((0, 384), (384, D)):
                        nc.vector.tensor_add(x2[:, s0:s1], ps_y[mo][:, s0:s1],
                                             x1b_t[mo][:, s0:s1])
                xo, xb = layernorm_one(x2[:], ln2w_bc, ln2b_bc, want_f32=debug)
                xb_t.append(xb)
                if debug and xo is not None:
                    nc.sync.dma_start(out=xdbg_d[i + 1, mo], in_=xo[:])

        # ---- classifier ----
        # per-m-tile end-to-end: m0's transposes -> w1 -> hTa -> logits ->
        # out-DMA start while m1's final LN2 still runs
        hTa_t = []
        for mo in range(MT):
            cxT = [xTp.tile([P, P], bf16, tag="xT", name="cxT")
                   for _ in range(KD)]
            for ko in range(KD):
                ptranspose(cxT[ko][:], xb_t[mo][:, ko * P:(ko + 1) * P])
            ps_h = psA.tile([P, 256], f32, tag="s", name="s")
            for ko in range(KD):
                nc.tensor.matmul(ps_h[0:100, 0:P], w1T_sb[:, ko, :], cxT[ko][:],
                                 start=(ko == 0), stop=(ko == KD - 1))
            hTa = const.tile([P, P], bf16, tag=f"hTa{mo}", name="hTa")
            nc.vector.memset(hTa[:, :], 1.0)
            nc.vector.tensor_copy(hTa[0:100, :], ps_h[0:100, 0:P])
            hTa_t.append(hTa)

        # logits in CH-wide groups: 4x512 psum chunks rotating through the
        # psC pool, evictions cast f32->bf16 alternating Scalar/Vector into
        # a staged [P, CH] tile, then one 512KB DMA per (group, m-tile) on
        # the sync queue (w2 residue streams on gpsimd concurrently).
        NSUB = CH // 512
        for ci in range(NCH):
            # residual w2 chunks interleave with out-writes on the sync
            # ring: single-queue serialization beats competing queues here
            load_w2(ci + w2p.bufs, eng=nc.sync)
            w2t = w2_tiles.pop(ci)
            for mo in range(MT):
                ost = ostp.tile([P, CH], bf16, tag="ost", name="ost")
                for sub in range(NSUB):
                    s0 = sub * 512
                    k = ci * MT * NSUB + mo * NSUB + sub
                    ps = psA.tile([P, 512], f32, tag="s", name="c")
                    nc.tensor.matmul(ps[:], hTa_t[mo][:],
                                     w2t[:, s0:s0 + 512], start=True, stop=True)
                    if k % 2 == 0:
                        nc.scalar.copy(ost[:, s0:s0 + 512], ps[:])
                    else:
                        nc.vector.tensor_copy(ost[:, s0:s0 + 512], ps[:])
                nc.sync.dma_start(out=out_d[ci, mo], in_=ost[:])

    nc.compile()
    return nc


CH = 2048
NCH = (NE + CH - 1) // CH


def _chunk_w2(cls_w2, cls_b2):
    # rows: 100 weights + 1 bias + 27 zero pad (lhsT rows 101.. are 1.0 from
    # the hTa memset, so the zero rows contribute nothing)
    w2a = np.concatenate(
        [cls_w2.T, cls_b2[None, :], np.zeros((27, NE), np.float32)], axis=0
    ).astype(BF16)  # [128, NE]
    pad = NCH * CH - NE
    if pad:
        w2a = np.concatenate([w2a, np.zeros((128, pad), BF16)], axis=1)
    return np.ascontiguousarray(w2a.reshape(128, NCH, CH).transpose(1, 0, 2))


def _prep(inputs):
    lhs = np.asarray(inputs["last_hidden_state"], dtype=np.float32)
    pos = np.asarray(inputs["entity_position_ids"])
    msk = np.asarray(inputs["entity_attention_mask"])
    qkv_w = np.asarray(inputs["qkv_w"], dtype=np.float32)
    qkv_b = np.asarray(inputs["qkv_b"], dtype=np.float32)
    out_w = np.asarray(inputs["out_w"], dtype=np.float32)
    out_b = np.asarray(inputs["out_b"], dtype=np.float32)
    ln1_w = np.asarray(inputs["ln1_w"], dtype=np.float32)
    ln1_b = np.asarray(inputs["ln1_b"], dtype=np.float32)
    ff1_w = np.asarray(inputs["ff1_w"], dtype=np.float32)
    ff1_b = np.asarray(inputs["ff1_b"], dtype=np.float32)
    ff2_w = np.asarray(inputs["ff2_w"], dtype=np.float32)
    ff2_b = np.asarray(inputs["ff2_b"], dtype=np.float32)
    ln2_w = np.asarray(inputs["ln2_w"], dtype=np.float32)
    ln2_b = np.asarray(inputs["ln2_b"], dtype=np.float32)
    cls_w1 = np.asarray(inputs["cls_w1"], dtype=np.float32)
    cls_w2 = np.asarray(inputs["cls_w2"], dtype=np.float32)
    cls_b2 = np.asarray(inputs["cls_b2"], dtype=np.float32)
    attn_w = np.asarray(inputs["attn_w"], dtype=np.float32)
    attn_b = float(np.asarray(inputs["attn_b"], dtype=np.float32))

    # ragged valid mask: 1 up to the first -1 (and only where attention mask set)
    nb = np.cumprod((pos != -1).astype(np.int32), axis=-1)
    valid = (msk != 0).astype(np.int32)[:, :, None] * nb       # [B, M, L]
    vmT = np.ascontiguousarray(valid.transpose(0, 2, 1)).astype(np.float32)

    cfg = (
        attn_b,
        bool(np.any(qkv_b[:, 2 * D:])),
        bool(np.any(out_b)),
        bool(np.any(ff2_b)),
        not (np.all(ln1_w == 1.0) and np.all(ln1_b == 0.0)),
        not (np.all(ln2_w == 1.0) and np.all(ln2_b == 0.0)),
        bool(KERNEL_DEBUG),
    )

    # qkvb rearranged host-side: [P, NL, 2*KD] where col (i, t) holds
    # qkv_b[i, t*P + p] for partition p; ff1b likewise [P, NL, KF]
    qkvb_r = np.ascontiguousarray(
        qkv_b[:, :2 * D].reshape(NL, 2 * KD, P).transpose(2, 0, 1))
    ff1b_r = np.ascontiguousarray(
        ff1_b.reshape(NL, KF, P).transpose(2, 0, 1))
    shared = {
        "attnw": np.ascontiguousarray(np.broadcast_to(attn_w, (L, D))),
        "qkvw": np.ascontiguousarray(qkv_w.transpose(0, 2, 1)).reshape(
            NL, KD, P, 3 * D).astype(BF16),
        "qkvb": qkvb_r,
        "outw": np.ascontiguousarray(out_w.transpose(0, 2, 1)).reshape(
            NL, KD, P, D).astype(BF16),
        "ff1w": np.ascontiguousarray(ff1_w.transpose(0, 2, 1)).reshape(
            NL, KD, P, DFF).astype(BF16),
        "ff1b": ff1b_r,
        "ff2w": np.ascontiguousarray(ff2_w.transpose(0, 2, 1)).reshape(
            NL, KF, P, D).astype(BF16),
        "w1T": np.ascontiguousarray(cls_w1.T).reshape(KD, P, 100).astype(BF16),
        "w2a": _chunk_w2(cls_w2, cls_b2),
    }
    if cfg[1]:
        shared["vbias"] = qkv_b[:, 2 * D:]
    if cfg[2]:
        shared["outb"] = out_b
    if cfg[3]:
        shared["ff2b"] = ff2_b
    if cfg[4]:
        shared["ln1w"] = ln1_w
        shared["ln1b"] = ln1_b
    if cfg[5]:
        shared["ln2w"] = ln2_w
        shared["ln2b"] = ln2_b

    lhs32 = np.ascontiguousarray(lhs[:, :L, :])
    in_maps = []
    for c in range(N_CORES):
        m = dict(shared)
        m["lhs32"] = np.ascontiguousarray(lhs32[c * BL:(c + 1) * BL])
        m["vmT"] = np.ascontiguousarray(vmT[c * BL:(c + 1) * BL])
        in_maps.append(m)
    return cfg, in_maps


def kernel(**inputs):
    from concourse.bass_utils import run_bass_kernel_spmd

    cfg, in_maps = _prep(inputs)
    if cfg not in _CACHE:
        _CACHE[cfg] = _build(cfg)
    nc = _CACHE[cfg]
    res = run_bass_kernel_spmd(nc, in_maps, core_ids=list(range(N_CORES)))
    # per-core out is [NCH, BL, P, CH] bf16 -> [BL, M, NE] f32
    outs = []
    for c in range(N_CORES):
        o = res.results[c]["out"]
        outs.append(o.transpose(1, 2, 0, 3).reshape(BL, M, NCH * CH)[:, :, :NE])
    out = np.concatenate(outs, axis=0).astype(np.float32)
    if KERNEL_DEBUG:
        kernel.last_debug = [res.results[c].get("xdbg") for c in range(N_CORES)]
    return out
